# revision 1
# baseline (speedup 1.0000x reference)
"""Causal multi-head attention on 8 Trainium2 NeuronCores.

Problem: Q,K,V [2,16,2048,128] f32, out = causal-softmax(QK^T/sqrt(128)) V.
Sharding: batch*heads = 32 slices -> 4 heads per core across 8 cores; each
core computes its heads fully independently (no collectives).

Per-head pipeline on one core (S=2048, D=128), all on HW fast paths:
  - Host pre-transposes Q,K per head to [128(d), 2048(seq)] and feeds them
    as float32r DRAM inputs (fp32r is full-rate on the PE at N>=256, and is
    only cheap when produced by DMA -- on-chip f32r producers lower to a
    pathologically slow rounding op).
  - Scores computed transposed, fp32r: S^T[k,q] = kt_tile.T @ qt_block into
    PSUM [128, 1024] (two 512-wide k-tiles per exp call).
  - Causal mask added in PSUM by a bf16 matmul: diag(-1e9) @ tri01 slice.
  - exp on ACT with fused scale=1/sqrt(128), PSUM -> SBUF, bf16 out (W^T).
  - PV direct in [q,d] layout: for each 128-query subtile,
    out[q, 0:129] += W^T[:, qsub].T @ [V|1] (bf16, N=129), accumulated over
    k-tiles in PSUM; column 128 accumulates the softmax denominators.
  - Normalize: DVE reciprocal of column 128, then tensor_scalar multiply
    fused with the PSUM->SBUF copy; one output DMA per head.
"""

import sys

sys.path.insert(0, "/opt/trn_rl_repo")

from contextlib import ExitStack

import numpy as np
import ml_dtypes

import concourse.bass as bass
import concourse.bacc as bacc
import concourse.mybir as mybir
import concourse.tile as tile

F32 = mybir.dt.float32
F32R = mybir.dt.float32r
BF16 = mybir.dt.bfloat16
F16 = mybir.dt.float16

B, H, S, D = 2, 16, 2048, 128
NCORES = 8
HPC = (B * H) // NCORES  # 4 heads per core
P = 128                  # partition dim / k-tile / q-subtile size
QB = 512                 # q block width (scores moving free dim)
NQB = S // QB            # 4
NKT = S // P             # 16 k-tiles per head
VW = 132                 # padded [V|1] row width (129 used)
SCALE = 1.0 / float(np.sqrt(128.0))
NEG = -1.0e9

Exp = mybir.ActivationFunctionType.Exp


def _emit_core(tc: tile.TileContext, ctx: ExitStack, qt_in, kt_in, v_in, o_out,
               diag_in, tri_in):
    nc = tc.nc

    const = ctx.enter_context(tc.tile_pool(name="const", bufs=1))
    big = ctx.enter_context(tc.tile_pool(name="big", bufs=2))
    wpool = ctx.enter_context(tc.tile_pool(name="w", bufs=3))
    small = ctx.enter_context(tc.tile_pool(name="small", bufs=4))
    ps_s = ctx.enter_context(tc.tile_pool(name="ps_s", bufs=2, space=bass.MemorySpace.PSUM))
    ps_o = ctx.enter_context(tc.tile_pool(name="ps_o", bufs=4, space=bass.MemorySpace.PSUM))

    diagneg = const.tile([P, P], BF16, tag="diagneg")
    tri = const.tile([P, 4 * QB], BF16, tag="tri")
    nc.sync.dma_start(diagneg[:], diag_in)
    nc.sync.dma_start(tri[:], tri_in)

    for h in range(HPC):
        # ---- load this head's tensors ----
        qt = big.tile([P, S], F32R, tag="qt")
        kt = big.tile([P, S], F32R, tag="kt")
        vf = big.tile([P, NKT, P], F32, tag="vf")
        nc.sync.dma_start(qt[:], qt_in[h])
        nc.sync.dma_start(kt[:], kt_in[h])
        nc.sync.dma_start(vf[:], v_in[h].rearrange("(t p) d -> p t d", p=P))
        vb = big.tile([P, NKT, VW], F16, tag="vb")
        nc.vector.tensor_copy(vb[:, :, 0:P], vf[:])
        nc.vector.memset(vb[:, :, P:P + 1], 1.0)

        outbuf = big.tile([P, NKT, P], F32, tag="outbuf")

        for qb in range(NQB):
            nkt = 4 * (qb + 1)  # causal: k-tiles 0..nkt-1
            po = []
            for _j in range(4):
                po_t = ps_o.tile([P, VW], F32, tag="po")
                po.append(po_t)
            for pair in range(nkt // 2):
                kts = (2 * pair, 2 * pair + 1)
                ps = ps_s.tile([P, 2 * QB], F32, tag="ps")
                for i, kkt in enumerate(kts):
                    r = kkt - 4 * qb
                    nc.tensor.matmul(ps[:, i * QB:(i + 1) * QB],
                                     kt[:, kkt * P:(kkt + 1) * P],
                                     qt[:, qb * QB:(qb + 1) * QB],
                                     start=True, stop=(r < 0))
                    if r >= 0:
                        ncols = (r + 1) * P
                        nc.tensor.matmul(ps[:, i * QB:i * QB + ncols],
                                         diagneg[:],
                                         tri[:, r * QB:r * QB + ncols],
                                         start=False, stop=True)
                w = wpool.tile([P, 2 * QB], F16, tag="w")
                nc.scalar.activation(w[:], ps[:], Exp, scale=SCALE)
                for i, kkt in enumerate(kts):
                    r = kkt - 4 * qb
                    for j in range(4):
                        if r > j:
                            continue  # query subtile fully masked for this k-tile
                        nc.tensor.matmul(po[j][:, 0:P + 1],
                                         w[:, i * QB + j * P:i * QB + (j + 1) * P],
                                         vb[:, kkt, 0:P + 1],
                                         start=(kkt == 0), stop=(kkt == 4 * qb + j))

            # ---- finalize this q block: normalize by the ones-column sums ----
            for j in range(4):
                rs = small.tile([P, 1], F32, tag="rs")
                nc.vector.reciprocal(rs[:], po[j][:, P:P + 1])
                qs = qb * 4 + j
                nc.vector.tensor_scalar_mul(outbuf[:, qs, :], po[j][:, 0:P], rs[:])

        nc.sync.dma_start(o_out[h].rearrange("(t p) d -> p t d", p=P), outbuf[:])


def build_nc():
    nc = bacc.Bacc("TRN2", target_bir_lowering=False, debug=False)
    qt = nc.dram_tensor("qt", [HPC, P, S], F32R, kind="ExternalInput")
    kt = nc.dram_tensor("kt", [HPC, P, S], F32R, kind="ExternalInput")
    v = nc.dram_tensor("v", [HPC, S, D], F32, kind="ExternalInput")
    diag = nc.dram_tensor("diagneg", [P, P], BF16, kind="ExternalInput")
    tri = nc.dram_tensor("tri", [P, 4 * QB], BF16, kind="ExternalInput")
    o = nc.dram_tensor("o", [HPC, S, D], F32, kind="ExternalOutput")
    with tile.TileContext(nc) as tc:
        with ExitStack() as ctx:
            _emit_core(tc, ctx, qt.ap(), kt.ap(), v.ap(), o.ap(),
                       diag.ap(), tri.ap())
    nc.compile()
    return nc


def make_consts():
    diag = (NEG * np.eye(P)).astype(ml_dtypes.bfloat16)
    tri = np.zeros((P, 4 * QB), dtype=ml_dtypes.bfloat16)
    for r in range(4):
        c = np.arange(P)[:, None]
        ql = np.arange(QB)[None, :]
        tri[:, r * QB:(r + 1) * QB] = (r * P + c > ql).astype(ml_dtypes.bfloat16)
    return diag, tri


def make_in_maps(Q, K, V):
    diag, tri = make_consts()
    Qr = np.asarray(Q, dtype=np.float32).reshape(B * H, S, D)
    Kr = np.asarray(K, dtype=np.float32).reshape(B * H, S, D)
    Vr = np.asarray(V, dtype=np.float32).reshape(B * H, S, D)
    QT = np.ascontiguousarray(Qr.transpose(0, 2, 1))  # [32, 128, 2048]
    KT = np.ascontiguousarray(Kr.transpose(0, 2, 1))
    in_maps = []
    for c in range(NCORES):
        sl = slice(c * HPC, (c + 1) * HPC)
        in_maps.append({
            "qt": QT[sl], "kt": KT[sl],
            "v": np.ascontiguousarray(Vr[sl]),
            "diagneg": diag, "tri": tri,
        })
    return in_maps


_NC = None


def kernel(Q: np.ndarray, K: np.ndarray, V: np.ndarray) -> np.ndarray:
    from concourse.bass_utils import run_bass_kernel_spmd

    global _NC
    if _NC is None:
        _NC = build_nc()
    nc = _NC

    in_maps = make_in_maps(Q, K, V)
    res = run_bass_kernel_spmd(nc, in_maps, core_ids=list(range(NCORES)))
    out = np.concatenate([res.results[c]["o"] for c in range(NCORES)], axis=0)
    return out.reshape(B, H, S, D).astype(np.float32)



# revision 3
# speedup vs baseline: 879.0486x; 879.0486x over previous
"""Causal multi-head attention on 8 Trainium2 NeuronCores.

Problem: Q,K,V [2,16,2048,128] f32, out = causal-softmax(QK^T/sqrt(128)) V.
Sharding: batch*heads = 32 slices -> 4 heads per core across 8 cores; each
core computes its heads fully independently (no collectives).

Per-head pipeline on one core (S=2048, D=128), all on HW fast paths:
  - Host pre-transposes Q,K per head to [128(d), 2048(seq)] and feeds them
    as float32r DRAM inputs (fp32r is full-rate on the PE at N>=256, and is
    only cheap when produced by DMA -- on-chip f32r producers lower to a
    pathologically slow rounding op).
  - Scores computed transposed, fp32r: S^T[k,q] = kt_tile.T @ qt_block into
    PSUM [128, 1024] (two 512-wide k-tiles per exp call).
  - Causal mask added in PSUM by a bf16 matmul: diag(-1e9) @ tri01 slice.
  - exp on ACT with fused scale=1/sqrt(128), PSUM -> SBUF, bf16 out (W^T).
  - PV direct in [q,d] layout: for each 128-query subtile,
    out[q, 0:129] += W^T[:, qsub].T @ [V|1] (bf16, N=129), accumulated over
    k-tiles in PSUM; column 128 accumulates the softmax denominators.
  - Normalize: DVE reciprocal of column 128, then tensor_scalar multiply
    fused with the PSUM->SBUF copy; one output DMA per head.
"""

import sys

sys.path.insert(0, "/opt/trn_rl_repo")

from contextlib import ExitStack

import numpy as np
import ml_dtypes

import concourse.bass as bass
import concourse.bacc as bacc
import concourse.mybir as mybir
import concourse.tile as tile

F32 = mybir.dt.float32
F32R = mybir.dt.float32r
BF16 = mybir.dt.bfloat16
F16 = mybir.dt.float16

B, H, S, D = 2, 16, 2048, 128
NCORES = 8
HPC = (B * H) // NCORES  # 4 heads per core
P = 128                  # partition dim / k-tile / q-subtile size
QB = 512                 # q block width (scores moving free dim)
NQB = S // QB            # 4
NKT = S // P             # 16 k-tiles per head
VW = 132                 # padded [V|1] row width (129 used)
SCALE = 1.0 / float(np.sqrt(128.0))
NEG = -1.0e9

Exp = mybir.ActivationFunctionType.Exp


def _emit_core(tc: tile.TileContext, ctx: ExitStack, qt_in, kt_in, v_in, o_out,
               diag_in, tri_in, repeat: int = 1):
    nc = tc.nc

    const = ctx.enter_context(tc.tile_pool(name="const", bufs=1))
    big = ctx.enter_context(tc.tile_pool(name="big", bufs=2))
    wpool = ctx.enter_context(tc.tile_pool(name="w", bufs=3))
    small = ctx.enter_context(tc.tile_pool(name="small", bufs=4))
    ps_s = ctx.enter_context(tc.tile_pool(name="ps_s", bufs=2, space=bass.MemorySpace.PSUM))
    ps_o = ctx.enter_context(tc.tile_pool(name="ps_o", bufs=4, space=bass.MemorySpace.PSUM))

    diagneg = const.tile([P, P], BF16, tag="diagneg")
    tri = const.tile([P, 4 * QB], BF16, tag="tri")
    nc.sync.dma_start(diagneg[:], diag_in)
    nc.sync.dma_start(tri[:], tri_in)

    loop_ctx = tc.For_i(0, repeat) if repeat > 1 else None
    if loop_ctx is not None:
        ctx.enter_context(loop_ctx)

    for h in range(HPC):
        # ---- load this head's tensors ----
        qt = big.tile([P, S], F32R, tag="qt")
        kt = big.tile([P, S], F32R, tag="kt")
        vf = big.tile([P, NKT, P], F32, tag="vf")
        nc.sync.dma_start(qt[:], qt_in[h])
        nc.sync.dma_start(kt[:], kt_in[h])
        nc.sync.dma_start(vf[:], v_in[h].rearrange("(t p) d -> p t d", p=P))
        vb = big.tile([P, NKT, VW], F16, tag="vb")
        nc.vector.tensor_copy(vb[:, :, 0:P], vf[:])
        nc.vector.memset(vb[:, :, P:P + 1], 1.0)

        outbuf = big.tile([P, NKT, P], F32, tag="outbuf")

        for qb in range(NQB):
            nkt = 4 * (qb + 1)  # causal: k-tiles 0..nkt-1
            po = []
            for _j in range(4):
                po_t = ps_o.tile([P, VW], F32, tag="po")
                po.append(po_t)
            for pair in range(nkt // 2):
                kts = (2 * pair, 2 * pair + 1)
                ps = ps_s.tile([P, 2 * QB], F32, tag="ps")
                for i, kkt in enumerate(kts):
                    r = kkt - 4 * qb
                    nc.tensor.matmul(ps[:, i * QB:(i + 1) * QB],
                                     kt[:, kkt * P:(kkt + 1) * P],
                                     qt[:, qb * QB:(qb + 1) * QB],
                                     start=True, stop=(r < 0))
                    if r >= 0:
                        ncols = (r + 1) * P
                        nc.tensor.matmul(ps[:, i * QB:i * QB + ncols],
                                         diagneg[:],
                                         tri[:, r * QB:r * QB + ncols],
                                         start=False, stop=True)
                w = wpool.tile([P, 2 * QB], F16, tag="w")
                nc.scalar.activation(w[:], ps[:], Exp, scale=SCALE)
                for i, kkt in enumerate(kts):
                    r = kkt - 4 * qb
                    for j in range(4):
                        if r > j:
                            continue  # query subtile fully masked for this k-tile
                        nc.tensor.matmul(po[j][:, 0:P + 1],
                                         w[:, i * QB + j * P:i * QB + (j + 1) * P],
                                         vb[:, kkt, 0:P + 1],
                                         start=(kkt == 0), stop=(kkt == 4 * qb + j))

            # ---- finalize this q block: normalize by the ones-column sums ----
            for j in range(4):
                rs = small.tile([P, 1], F32, tag="rs")
                nc.vector.reciprocal(rs[:], po[j][:, P:P + 1])
                qs = qb * 4 + j
                nc.vector.tensor_scalar_mul(outbuf[:, qs, :], po[j][:, 0:P], rs[:])

        nc.sync.dma_start(o_out[h].rearrange("(t p) d -> p t d", p=P), outbuf[:])


def build_nc(repeat: int = 1):
    nc = bacc.Bacc("TRN2", target_bir_lowering=False, debug=False)
    qt = nc.dram_tensor("qt", [HPC, P, S], F32R, kind="ExternalInput")
    kt = nc.dram_tensor("kt", [HPC, P, S], F32R, kind="ExternalInput")
    v = nc.dram_tensor("v", [HPC, S, D], F32, kind="ExternalInput")
    diag = nc.dram_tensor("diagneg", [P, P], BF16, kind="ExternalInput")
    tri = nc.dram_tensor("tri", [P, 4 * QB], BF16, kind="ExternalInput")
    o = nc.dram_tensor("o", [HPC, S, D], F32, kind="ExternalOutput")
    with tile.TileContext(nc) as tc:
        with ExitStack() as ctx:
            _emit_core(tc, ctx, qt.ap(), kt.ap(), v.ap(), o.ap(),
                       diag.ap(), tri.ap(), repeat=repeat)
    nc.compile()
    return nc


def make_consts():
    diag = (NEG * np.eye(P)).astype(ml_dtypes.bfloat16)
    tri = np.zeros((P, 4 * QB), dtype=ml_dtypes.bfloat16)
    for r in range(4):
        c = np.arange(P)[:, None]
        ql = np.arange(QB)[None, :]
        tri[:, r * QB:(r + 1) * QB] = (r * P + c > ql).astype(ml_dtypes.bfloat16)
    return diag, tri


def make_in_maps(Q, K, V):
    diag, tri = make_consts()
    Qr = np.asarray(Q, dtype=np.float32).reshape(B * H, S, D)
    Kr = np.asarray(K, dtype=np.float32).reshape(B * H, S, D)
    Vr = np.asarray(V, dtype=np.float32).reshape(B * H, S, D)
    QT = np.ascontiguousarray(Qr.transpose(0, 2, 1))  # [32, 128, 2048]
    KT = np.ascontiguousarray(Kr.transpose(0, 2, 1))
    in_maps = []
    for c in range(NCORES):
        sl = slice(c * HPC, (c + 1) * HPC)
        in_maps.append({
            "qt": QT[sl], "kt": KT[sl],
            "v": np.ascontiguousarray(Vr[sl]),
            "diagneg": diag, "tri": tri,
        })
    return in_maps


_NC = None


def kernel(Q: np.ndarray, K: np.ndarray, V: np.ndarray) -> np.ndarray:
    from concourse.bass_utils import run_bass_kernel_spmd

    global _NC
    if _NC is None:
        _NC = build_nc()
    nc = _NC

    in_maps = make_in_maps(Q, K, V)
    res = run_bass_kernel_spmd(nc, in_maps, core_ids=list(range(NCORES)))
    out = np.concatenate([res.results[c]["o"] for c in range(NCORES)], axis=0)
    return out.reshape(B, H, S, D).astype(np.float32)

